# revision 20
# baseline (speedup 1.0000x reference)
"""Causal self-attention (B=2, S=4096, D=768, H=12) on 8 Trainium2 NeuronCores.

Sharding: data + head parallel. Core c handles batch c//4 and the 3 heads
starting at (c%4)*3. Each core computes the qkv projection for its heads,
causal attention, and a partial output projection (its heads' rows of w_out);
the host sums the 4 partial outputs per batch (bf16 partials, fp32 sum).

Device design notes:
 - x arrives pre-transposed (xT [768, 4096]) so the contraction dim lands on
   SBUF partitions for every projection matmul.
 - q, k are produced transposed; scores are computed transposed ([sk, sq]) so
   the PV matmul consumes exp(scores) directly as the moving operand; a
   ones-column appended to v yields softmax denominators for free.
 - Every head uses the alternating-row-half layout: q^T duplicated on both
   partition halves, k^T chunks folded onto alternating halves, so consecutive
   QK matmuls hit distinct PE row groups and co-execute.
 - Heads are processed sequentially per query block, so only 2 PSUM ctx banks
   are needed; PSUM plan: scores 2x[128,1024], ctx 2x[128,512], proj 2x[128,512].
 - exp: ScalarE activation for 2/3 of score groups; the rest on VectorE via a
   Schraudolph bit-trick (bf16 bits = round(23.083*s + 16250.5) as int16),
   keeping both engines below the TensorE critical path.
 - Projection/out-projection accumulation chains are built pairwise on the two
   proj PSUM banks and their links woven between attention matmuls so PSUM
   bank read-modify-write never serializes back-to-back matmuls.
 - Causal masking: GPSIMD affine_select (predicated fill) on exp(scores) for
   the 4 diagonal-chunk patterns only.
"""

import numpy as np

try:
    import concourse.bass as bass  # noqa: F401
except ImportError:
    import sys
    sys.path.insert(0, "/opt/trn_rl_repo")

import concourse.bass as bass
import concourse.tile as tile
from concourse import bacc, mybir
from concourse.bass_utils import run_bass_kernel_spmd

F32 = mybir.dt.float32
BF16 = mybir.dt.bfloat16
I16 = mybir.dt.int16
N_CORES = 8
B, S, D, H, HD = 2, 4096, 768, 12, 64
HPC = 3            # heads per core
SC = 512           # sequence chunk (free dim of most matmuls)
NSC = S // SC      # 8
KC = 128           # contraction chunk
NKC = D // KC      # 6
NQB = S // SC      # query blocks of 512
CPB = SC // KC     # key chunks per query block (4)
VW = HPC * (HD + 1)  # 195 v columns per key-chunk: [64 v | 1] x 3 heads

# Schraudolph exp in bf16-bits domain: bits(exp(0.125*s)) ~= A*s + B
A_DVE = (128.0 / float(np.log(2.0))) * 0.125
B_DVE = 16256.0 - 5.5

_CACHE = {}


def _emit(nc, tc, ins, out_ap):
    xT, wqk, wv, wo = ins
    MM = nc.tensor.matmul
    GE = mybir.AluOpType.is_ge

    constp = tc.alloc_tile_pool(name="const", bufs=1)
    xtp = tc.alloc_tile_pool(name="xt", bufs=12)
    q2p = tc.alloc_tile_pool(name="q2p", bufs=3 * NSC)
    k2p = tc.alloc_tile_pool(name="k2p", bufs=3 * NSC)
    vp = tc.alloc_tile_pool(name="vp", bufs=NSC)
    ctxp = tc.alloc_tile_pool(name="ctx", bufs=2 * NQB)
    expp = tc.alloc_tile_pool(name="exp", bufs=6)
    smp = tc.alloc_tile_pool(name="sm", bufs=3)
    ostp = tc.alloc_tile_pool(name="ost", bufs=3)
    psSG = tc.alloc_tile_pool(name="psSG", bufs=2, space="PSUM")   # 2x2 banks
    psCTX = tc.alloc_tile_pool(name="psCTX", bufs=2, space="PSUM")  # 2x1 banks
    psPR = tc.alloc_tile_pool(name="psPR", bufs=2, space="PSUM")   # 2x1 banks

    # ---- constants (DMAs issued later, interleaved with x for fast startup) ----
    wqk_sb = constp.tile([128, NKC * 384], BF16, tag="wqk")
    wv_sb = constp.tile([128, NKC * 256], BF16, tag="wv")
    woAB_sb = constp.tile([128, 768], BF16, tag="woAB")
    woC_sb = constp.tile([64, 768], BF16, tag="woC")

    # persistent activations
    # qAB/kAB: heads A,B stacked on row halves, keys/queries contiguous.
    # q2/k2 (head C only): q^T duplicated on both row halves; k^T chunks
    # folded: ck -> rows (ck%2)*64, cols ((ck%4)//2)*128 — so consecutive
    # head-C QK matmuls alternate PE row groups like the A/B pairs do.
    qAB = [q2p.tile([128, SC], BF16, tag="qAB", name=f"qAB{i}") for i in range(NSC)]
    kAB = [k2p.tile([128, SC], BF16, tag="kAB", name=f"kAB{i}") for i in range(NSC)]
    q2 = [q2p.tile([128, SC], BF16, tag="q2", name=f"q2C_{i}") for i in range(NSC)]
    k2 = [k2p.tile([128, 256], BF16, tag="k2", name=f"k2C_{i}") for i in range(NSC)]
    vt = [vp.tile([128, CPB * VW], BF16, tag="v", name=f"vt{i}") for i in range(NSC)]
    ctxAB = [ctxp.tile([128, SC], BF16, tag="ctxAB", name=f"ctxAB{i}")
             for i in range(NQB)]
    ctxC = [ctxp.tile([64, SC], BF16, tag="ctxC", name=f"ctxC{i}")
            for i in range(NQB)]

    # ---- extras machinery (proj / outproj work woven between attention) ----
    xts = {}
    extras = []
    pulled = [0]

    def pull():
        if pulled[0] < len(extras):
            extras[pulled[0]]()
            pulled[0] += 1

    def dma_piece(sc):
        def f():
            xts[sc] = []
            for k in range(NKC):
                xt = xtp.tile([128, SC], BF16, tag="xt", name=f"xt{k}_{sc}")
                nc.sync.dma_start(xt[:], xT[k * 128:(k + 1) * 128, sc * SC:(sc + 1) * SC])
                xts[sc].append(xt)
        return f

    def k_fold(sc, src):
        # src [64,512] psum: cols = ck*128+c -> k2 rows (ck%2)*64, cols (ck//2)*128+c
        s_r = src.rearrange("p (g t c) -> p g t c", g=2, t=2, c=128)
        d_lo = k2[sc][0:64, :].rearrange("p (g c) -> p g c", g=2, c=128)
        d_hi = k2[sc][64:128, :].rearrange("p (g c) -> p g c", g=2, c=128)
        nc.vector.tensor_copy(d_lo, s_r[:, :, 0, :])
        nc.vector.tensor_copy(d_hi, s_r[:, :, 1, :])

    def m_chain(sc, m):
        st = {}
        links = []
        for k in range(NKC):
            def link(k=k):
                if k == 0:
                    st["ps"] = psPR.tile([128, SC], F32, tag="pr", name=f"prm{sc}_{m}")
                MM(st["ps"][:], wqk_sb[:, k * 384 + m * 128: k * 384 + (m + 1) * 128],
                   xts[sc][k][:], start=(k == 0), stop=(k == NKC - 1))
            links.append(link)

        def copy():
            ps = st["ps"]
            if m == 0:
                nc.vector.tensor_copy(qAB[sc][:], ps[:])
            elif m == 1:
                nc.vector.tensor_copy(q2[sc][0:64, :], ps[0:64, :])
                nc.vector.tensor_copy(q2[sc][64:128, :], ps[0:64, :])
                k_fold(sc, ps[64:128, :])
            else:
                nc.vector.tensor_copy(kAB[sc][:], ps[:])
        return links, copy

    def v_chain(sc, j):
        st = {}
        links = []
        for k in range(NKC):
            def link(k=k):
                if k == 0:
                    st["pv"] = psPR.tile([128, SC], F32, tag="pr", name=f"prv{sc}_{j}")
                MM(st["pv"][:, 0:256], xts[sc][k][:, j * 128:(j + 1) * 128],
                   wv_sb[:, k * 256:(k + 1) * 256], start=(k == 0), stop=(k == NKC - 1))
            links.append(link)

        def copy():
            v_r = vt[sc][:].rearrange("p (c h e) -> p c h e", h=HPC, e=HD + 1)
            nc.vector.tensor_copy(
                v_r[:, j, :, 0:HD],
                st["pv"][:, 0:HPC * HD].rearrange("p (h e) -> p h e", e=HD))
        return links, copy

    def po_chain(sc2, half):
        qb_ = sc2 // CPB
        csl = slice((sc2 % CPB) * 128, (sc2 % CPB + 1) * 128)
        w0, w1 = (0, 512) if half == 0 else (512, 768)
        n = w1 - w0
        st = {}
        links = []

        def l0():
            st["po"] = psPR.tile([128, SC], F32, tag="pr", name=f"po{sc2}_{half}")
            MM(st["po"][:, 0:n], ctxAB[qb_][:, csl], woAB_sb[:, w0:w1],
               start=True, stop=False)

        def l1():
            MM(st["po"][:, 0:n], ctxC[qb_][:, csl], woC_sb[:, w0:w1],
               start=False, stop=True)
        links += [l0, l1]
        return links, st

    def outproj_pieces(sc2):
        # two paired 2-link chains (col halves) + cast + dma
        la, sta = po_chain(sc2, 0)
        lb, stb = po_chain(sc2, 1)
        pieces = [lambda: (la[0](), lb[0]()), lambda: (la[1](), lb[1]())]

        def cast_dma():
            ost = ostp.tile([128, 768], BF16, tag="ost", name=f"ost{sc2}")
            nc.vector.tensor_copy(ost[:, 0:512], sta["po"][:, 0:512])
            nc.vector.tensor_copy(ost[:, 512:768], stb["po"][:, 0:256])
            nc.sync.dma_start(out_ap[sc2 * 128:(sc2 + 1) * 128, :], ost[:])
        pieces.append(cast_dma)
        return pieces

    def pair_chains(chain_list):
        # chain_list: [(links, copy), ...] -> pieces, links of adjacent chains
        # interleaved so consecutive matmuls alternate the two psPR banks.
        pieces = []
        for i in range(0, len(chain_list), 2):
            pair = chain_list[i:i + 2]
            if len(pair) == 2:
                (lA, cA), (lB, cB) = pair
                for a, b in zip(lA, lB):
                    pieces.append(lambda a=a, b=b: (a(), b()))
                pieces.append(cA)
                pieces.append(cB)
            else:
                (lA, cA), = pair
                for a in lA:
                    pieces.append(a)
                pieces.append(cA)
        return pieces

    def proj_pieces(sc):
        chains = ([m_chain(sc, m) for m in range(3)] +
                  [v_chain(sc, j) for j in range(CPB)])
        return pair_chains(chains)

    # ---- attention ----
    pend = []
    gctr = [0]
    cps = {}

    def norm_head(qb, h, num, den):
        # reciprocal_approx_fast is a bitwise-seed op and misreads PSUM;
        # bounce the denominator row through SBUF first
        dn = smp.tile([1, SC], F32, tag="dn", name=f"dn{h}_{qb}")
        nc.vector.tensor_copy(dn[:], den)
        rec = smp.tile([1, SC], F32, tag="rec", name=f"rec{h}_{qb}")
        nc.vector.reciprocal_approx_fast(rec[:], dn[:])
        bc = smp.tile([64, SC], F32, tag="bc", name=f"bc{h}_{qb}")
        nc.gpsimd.partition_broadcast(bc[:], rec[:])
        if h == 0:
            nc.vector.tensor_mul(ctxAB[qb][0:64, :], num, bc[:])
        elif h == 1:
            nc.vector.tensor_mul(ctxAB[qb][64:128, :], num, bc[:])
        else:
            nc.vector.tensor_mul(ctxC[qb][:], num, bc[:])

    def norm01(qb):
        norm_head(qb, 0, cps[0][0:HD, :], cps[0][HD:HD + 1, :])
        norm_head(qb, 1, cps[1][0:HD, :], cps[1][HD:HD + 1, :])

    def norm2(qb):
        norm_head(qb, 2, cps[2][0:HD, :], cps[2][HD:HD + 1, :])

    def emit_pv(weave=True):
        kind, qb, ck0, eg, last = pend.pop(0)
        nch = (qb + 1) * CPB
        if ck0 == 0:
            # allocate ctx accumulators at first PV pop, not at QK emission:
            # the pool ring has 2 bufs and the previous phase's lagged PV
            # writes must all be emitted before its banks are recycled
            if kind == "01":
                cps[0] = psCTX.tile([128, SC], F32, tag="cps", name=f"cps0_{qb}")
                cps[1] = psCTX.tile([128, SC], F32, tag="cps", name=f"cps1_{qb}")
            else:
                cps[2] = psCTX.tile([128, SC], F32, tag="cps", name=f"cps2_{qb}")
        for si in (0, 1):
            if kind == "01":
                h, ck = si, ck0
            else:
                h, ck = 2, ck0 + si
            o = max(0, (ck - (nch - CPB)) * 128)  # causal q-range restriction
            egv = eg[:, si * SC + o:(si + 1) * SC].bitcast(BF16)
            if ck >= nch - CPB:  # diagonal chunk: mask leading cols of view
                nc.gpsimd.affine_select(
                    egv[:, 0:128], egv[:, 0:128], pattern=[[1, 128]],
                    compare_op=GE, fill=0.0, base=0, channel_multiplier=-1)
            v_r = vt[ck // CPB][:].rearrange("p (c h e) -> p c h e",
                                             h=HPC, e=HD + 1)
            dst = cps[h]
            MM(dst[0:HD + 1, o:SC], v_r[:, ck % CPB, h, :], egv,
               start=(ck == 0), stop=(ck == nch - 1))
            if si == 0 and weave:
                pull()
        if last:
            norm01(qb) if kind == "01" else norm2(qb)

    def group01(qb, ck):
        nch = (qb + 1) * CPB

        def f():
            sg = psSG.tile([128, 2 * SC], F32, tag="sg", name=f"sg{qb}_01_{ck}")
            eg = expp.tile([128, 2 * SC], I16, tag="eg", name=f"eg{qb}_01_{ck}")
            o = max(0, (ck - (nch - CPB)) * 128)
            cg = (ck % CPB) * 128
            for h in (0, 1):
                MM(sg[:, h * SC + o:(h + 1) * SC],
                   kAB[ck // CPB][h * 64:(h + 1) * 64, cg:cg + 128],
                   qAB[qb][h * 64:(h + 1) * 64, o:SC], start=True, stop=True)
            _exp(sg, eg, o)
            pend.append(("01", qb, ck, eg, ck == nch - 1))
            if len(pend) > 2:
                emit_pv()
        return f

    def group2(qb, ck0):
        nch = (qb + 1) * CPB

        def f():
            sg = psSG.tile([128, 2 * SC], F32, tag="sg", name=f"sg{qb}_2_{ck0}")
            eg = expp.tile([128, 2 * SC], I16, tag="eg", name=f"eg{qb}_2_{ck0}")
            for si, ck in enumerate((ck0, ck0 + 1)):
                o = max(0, (ck - (nch - CPB)) * 128)
                if ck == 1 and nch == CPB:
                    o = 0
                rh = (ck % 2) * 64
                cg = ((ck % CPB) // 2) * 128
                MM(sg[:, si * SC + o:(si + 1) * SC],
                   k2[ck // CPB][rh:rh + 64, cg:cg + 128],
                   q2[qb][rh:rh + 64, o:SC], start=True, stop=True)
            o0 = max(0, (ck0 - (nch - CPB)) * 128)
            _exp(sg, eg, o0)
            pend.append(("2", qb, ck0, eg, ck0 == nch - 2))
            if len(pend) > 2:
                emit_pv()
        return f

    def _exp(sg, eg, o0):
        if gctr[0] % 5 == 4:
            nc.vector.tensor_scalar(eg[:, o0:], sg[:, o0:], A_DVE, B_DVE,
                                    mybir.AluOpType.mult, mybir.AluOpType.add)
        else:
            nc.scalar.activation(eg[:, o0:].bitcast(BF16), sg[:, o0:],
                                 mybir.ActivationFunctionType.Exp, scale=0.125)
        gctr[0] += 1

    # ---- emission ----
    # startup: interleave the wqk/x-chunk-0 DMAs so the first projection
    # chain can begin after the first pair lands; q/k chains first so qb0
    # attention can begin while v chains run.
    xts[0] = []
    for k in range(NKC):
        nc.sync.dma_start(wqk_sb[:, k * 384:(k + 1) * 384],
                          wqk[k * 128:(k + 1) * 128, :])
        xt = xtp.tile([128, SC], BF16, tag="xt", name=f"xt{k}_0")
        nc.sync.dma_start(xt[:], xT[k * 128:(k + 1) * 128, 0:SC])
        xts[0].append(xt)
    mq, cq = m_chain(0, 0)
    mk, ck_ = m_chain(0, 2)
    for piece in pair_chains([(mq, cq), (mk, ck_)]):
        piece()
    for k in range(NKC):
        nc.sync.dma_start(wv_sb[:, k * 256:(k + 1) * 256],
                          wv[k * 128:(k + 1) * 128, :])
    dma_piece(1)()
    nc.sync.dma_start(woAB_sb[:], wo[0:128, :])
    nc.sync.dma_start(woC_sb[:], wo[128:192, :])
    ones_st = smp.tile([128, CPB], F32, tag="ones")
    nc.vector.memset(ones_st[:], 1.0)
    for i in range(NSC):
        v_r = vt[i][:].rearrange("p (c h e) -> p c h e", h=HPC, e=HD + 1)
        for h in range(HPC):
            nc.vector.tensor_copy(v_r[:, :, h, HD], ones_st[:])
    for piece in pair_chains([m_chain(0, 1)] + [v_chain(0, j) for j in range(CPB)]):
        piece()

    for qb in range(NQB):
        nch = (qb + 1) * CPB
        extras = []
        pulled[0] = 0
        if qb + 2 < NSC:
            extras.append(dma_piece(qb + 2))
        if qb + 1 < NSC:
            extras += proj_pieces(qb + 1)
        if qb >= 1:
            for j in range(CPB):
                extras += outproj_pieces((qb - 1) * CPB + j)
        groups = []
        for ck in range(nch):
            groups.append(group01(qb, ck))
        for g in range(nch // 2):
            groups.append(group2(qb, 2 * g))
        n, k0 = len(groups), len(extras)
        for i, gth in enumerate(groups):
            gth()
            due = (i + 1) * k0 // n
            while pulled[0] < due:
                pull()
        while pend:
            emit_pv()
        while pulled[0] < k0:
            pull()

    extras = []
    pulled[0] = 0
    for j in range(CPB):
        extras += outproj_pieces((NQB - 1) * CPB + j)
    for piece in extras:
        piece()

    for p in (psPR, psCTX, psSG, ostp, smp, expp, ctxp, vp, k2p, q2p, xtp, constp):
        p.release()


def _build():
    if "nc" in _CACHE:
        return _CACHE["nc"]
    nc = bacc.Bacc("TRN2", target_bir_lowering=False, debug=False, num_devices=N_CORES)
    xT = nc.dram_tensor("xT", [D, S], BF16, kind="ExternalInput").ap()
    wqk = nc.dram_tensor("wqk", [D, 384], BF16, kind="ExternalInput").ap()
    wv = nc.dram_tensor("wv", [D, 256], BF16, kind="ExternalInput").ap()
    wo = nc.dram_tensor("wo", [HPC * HD, D], BF16, kind="ExternalInput").ap()
    out = nc.dram_tensor("out", [S, D], BF16, kind="ExternalOutput").ap()
    with tile.TileContext(nc) as tc:
        _emit(nc, tc, (xT, wqk, wv, wo), out)
    nc.compile()
    _CACHE["nc"] = nc
    return nc


def _in_maps(x, w_qkv, w_out):
    import ml_dtypes
    xTs = [np.ascontiguousarray(x[b].T).astype(ml_dtypes.bfloat16) for b in range(B)]
    maps = []
    for c in range(N_CORES):
        b = c // 4
        h0 = (c % 4) * HPC
        cols = lambda base, h: w_qkv[:, base + (h0 + h) * HD: base + (h0 + h + 1) * HD]
        wqk = np.ascontiguousarray(np.concatenate(
            [cols(0, 0), cols(0, 1),            # m0: qA | qB
             cols(0, 2), cols(D, 2),            # m1: qC | kC
             cols(D, 0), cols(D, 1)], axis=1)).astype(ml_dtypes.bfloat16)
        wv = np.ascontiguousarray(np.concatenate(
            [cols(2 * D, 0), cols(2 * D, 1), cols(2 * D, 2),
             np.zeros((D, 64), np.float32)], axis=1)).astype(ml_dtypes.bfloat16)
        wo = np.ascontiguousarray(
            w_out[h0 * HD:(h0 + HPC) * HD, :]).astype(ml_dtypes.bfloat16)
        maps.append({"xT": xTs[b], "wqk": wqk, "wv": wv, "wo": wo})
    return maps


def run_sharded(x, w_qkv, w_out, **spmd_kwargs):
    nc = _build()
    res = run_bass_kernel_spmd(nc, _in_maps(x, w_qkv, w_out),
                               list(range(N_CORES)), **spmd_kwargs)
    outs = [res.results[c]["out"].astype(np.float32) for c in range(N_CORES)]
    y = np.empty((B, S, D), np.float32)
    for b in range(B):
        y[b] = outs[4 * b] + outs[4 * b + 1] + outs[4 * b + 2] + outs[4 * b + 3]
    return y, res


def kernel(x, w_qkv, w_out):
    x = np.asarray(x, dtype=np.float32)
    w_qkv = np.asarray(w_qkv, dtype=np.float32)
    w_out = np.asarray(w_out, dtype=np.float32)
    y, _ = run_sharded(x, w_qkv, w_out)
    return y


# revision 25
# speedup vs baseline: 1.0449x; 1.0449x over previous
"""Causal self-attention (B=2, S=4096, D=768, H=12) on 8 Trainium2 NeuronCores.

Sharding: data + head parallel. Core c handles batch c//4 and the 3 heads
starting at (c%4)*3. Each core computes the qkv projection for its heads,
causal attention, and a partial output projection (its heads' rows of w_out);
the host sums the 4 partial outputs per batch (bf16 partials, fp32 sum).

Device design notes:
 - x arrives pre-transposed (xT [768, 4096]) so the contraction dim lands on
   SBUF partitions for every projection matmul.
 - q, k are produced transposed; scores are computed transposed ([sk, sq]) so
   the PV matmul consumes exp(scores) directly as the moving operand; a
   ones-column appended to v yields softmax denominators for free.
 - Every head uses the alternating-row-half layout: q^T duplicated on both
   partition halves, k^T chunks folded onto alternating halves, so consecutive
   QK matmuls hit distinct PE row groups and co-execute.
 - Heads are processed sequentially per query block, so only 2 PSUM ctx banks
   are needed; PSUM plan: scores 2x[128,1024], ctx 2x[128,512], proj 2x[128,512].
 - exp: ScalarE activation for 2/3 of score groups; the rest on VectorE via a
   Schraudolph bit-trick (bf16 bits = round(23.083*s + 16250.5) as int16),
   keeping both engines below the TensorE critical path.
 - Projection/out-projection accumulation chains are built pairwise on the two
   proj PSUM banks and their links woven between attention matmuls so PSUM
   bank read-modify-write never serializes back-to-back matmuls.
 - Causal masking: GPSIMD affine_select (predicated fill) on exp(scores) for
   the 4 diagonal-chunk patterns only.
"""

import numpy as np

try:
    import concourse.bass as bass  # noqa: F401
except ImportError:
    import sys
    sys.path.insert(0, "/opt/trn_rl_repo")

import concourse.bass as bass
import concourse.tile as tile
from concourse import bacc, mybir
from concourse.bass_utils import run_bass_kernel_spmd

F32 = mybir.dt.float32
BF16 = mybir.dt.bfloat16
I16 = mybir.dt.int16
N_CORES = 8
B, S, D, H, HD = 2, 4096, 768, 12, 64
HPC = 3            # heads per core
SC = 512           # sequence chunk (free dim of most matmuls)
NSC = S // SC      # 8
KC = 128           # contraction chunk
NKC = D // KC      # 6
NQB = S // SC      # query blocks of 512
CPB = SC // KC     # key chunks per query block (4)
VW = HPC * (HD + 1)  # 195 v columns per key-chunk: [64 v | 1] x 3 heads

# Schraudolph exp in bf16-bits domain: bits(exp(0.125*s)) ~= A*s + B
A_DVE = (128.0 / float(np.log(2.0))) * 0.125
B_DVE = 16256.0 - 5.5

_CACHE = {}


def _emit(nc, tc, ins, out_ap):
    xT, wqk, wv, wo = ins
    MM = nc.tensor.matmul
    GE = mybir.AluOpType.is_ge

    constp = tc.alloc_tile_pool(name="const", bufs=1)
    xtp = tc.alloc_tile_pool(name="xt", bufs=12)
    q2p = tc.alloc_tile_pool(name="q2p", bufs=3 * NSC)
    k2p = tc.alloc_tile_pool(name="k2p", bufs=3 * NSC)
    vp = tc.alloc_tile_pool(name="vp", bufs=NSC)
    ctxp = tc.alloc_tile_pool(name="ctx", bufs=2 * NQB)
    expp = tc.alloc_tile_pool(name="exp", bufs=6)
    smp = tc.alloc_tile_pool(name="sm", bufs=3)
    ostp = tc.alloc_tile_pool(name="ost", bufs=3)
    psSG = tc.alloc_tile_pool(name="psSG", bufs=2, space="PSUM")   # 2x2 banks
    psCTX = tc.alloc_tile_pool(name="psCTX", bufs=2, space="PSUM")  # 2x1 banks
    psPR = tc.alloc_tile_pool(name="psPR", bufs=2, space="PSUM")   # 2x1 banks

    # ---- constants (DMAs issued later, interleaved with x for fast startup) ----
    wqk_sb = constp.tile([128, NKC * 384], BF16, tag="wqk")
    wv_sb = constp.tile([128, NKC * 256], BF16, tag="wv")
    woAB_sb = constp.tile([128, 768], BF16, tag="woAB")
    woC_sb = constp.tile([64, 768], BF16, tag="woC")

    # persistent activations
    # q2[h][sc]: q^T duplicated on both row halves [128, 512]
    # k2[h][sc]: k^T chunks folded: ck -> rows (ck%2)*64, cols ((ck%4)//2)*128
    # so consecutive QK matmuls of one head alternate PE row groups.
    q2 = [[q2p.tile([128, SC], BF16, tag="q2", name=f"q2_{h}_{i}")
           for i in range(NSC)] for h in range(HPC)]
    k2 = [[k2p.tile([128, 256], BF16, tag="k2", name=f"k2_{h}_{i}")
           for i in range(NSC)] for h in range(HPC)]
    vt = [vp.tile([128, CPB * VW], BF16, tag="v", name=f"vt{i}") for i in range(NSC)]
    ctxAB = [ctxp.tile([128, SC], BF16, tag="ctxAB", name=f"ctxAB{i}")
             for i in range(NQB)]
    ctxC = [ctxp.tile([64, SC], BF16, tag="ctxC", name=f"ctxC{i}")
            for i in range(NQB)]

    # ---- extras machinery (proj / outproj work woven between attention) ----
    xts = {}
    extras = []
    pulled = [0]

    def pull():
        if pulled[0] < len(extras):
            extras[pulled[0]]()
            pulled[0] += 1

    def dma_piece(sc):
        def f():
            xts[sc] = []
            for k in range(NKC):
                xt = xtp.tile([128, SC], BF16, tag="xt", name=f"xt{k}_{sc}")
                nc.sync.dma_start(xt[:], xT[k * 128:(k + 1) * 128, sc * SC:(sc + 1) * SC])
                xts[sc].append(xt)
        return f

    def k_fold(h, sc, src):
        # src [64,512] psum: cols = ck*128+c -> k2 rows (ck%2)*64, cols (ck//2)*128+c
        s_r = src.rearrange("p (g t c) -> p g t c", g=2, t=2, c=128)
        d_lo = k2[h][sc][0:64, :].rearrange("p (g c) -> p g c", g=2, c=128)
        d_hi = k2[h][sc][64:128, :].rearrange("p (g c) -> p g c", g=2, c=128)
        nc.vector.tensor_copy(d_lo, s_r[:, :, 0, :])
        nc.vector.tensor_copy(d_hi, s_r[:, :, 1, :])

    def m_chain(sc, m):
        st = {}
        links = []
        for k in range(NKC):
            def link(k=k):
                if k == 0:
                    st["ps"] = psPR.tile([128, SC], F32, tag="pr", name=f"prm{sc}_{m}")
                MM(st["ps"][:], wqk_sb[:, k * 384 + m * 128: k * 384 + (m + 1) * 128],
                   xts[sc][k][:], start=(k == 0), stop=(k == NKC - 1))
            links.append(link)

        def copy():
            ps = st["ps"]
            if m == 0:
                nc.vector.tensor_copy(q2[0][sc][0:64, :], ps[0:64, :])
                nc.vector.tensor_copy(q2[0][sc][64:128, :], ps[0:64, :])
                nc.vector.tensor_copy(q2[1][sc][0:64, :], ps[64:128, :])
                nc.vector.tensor_copy(q2[1][sc][64:128, :], ps[64:128, :])
            elif m == 1:
                nc.vector.tensor_copy(q2[2][sc][0:64, :], ps[0:64, :])
                nc.vector.tensor_copy(q2[2][sc][64:128, :], ps[0:64, :])
                k_fold(2, sc, ps[64:128, :])
            else:
                k_fold(0, sc, ps[0:64, :])
                k_fold(1, sc, ps[64:128, :])
        return links, copy

    def v_chain(sc, j):
        st = {}
        links = []
        for k in range(NKC):
            def link(k=k):
                if k == 0:
                    st["pv"] = psPR.tile([128, SC], F32, tag="pr", name=f"prv{sc}_{j}")
                MM(st["pv"][:, 0:256], xts[sc][k][:, j * 128:(j + 1) * 128],
                   wv_sb[:, k * 256:(k + 1) * 256], start=(k == 0), stop=(k == NKC - 1))
            links.append(link)

        def copy():
            v_r = vt[sc][:].rearrange("p (c h e) -> p c h e", h=HPC, e=HD + 1)
            nc.vector.tensor_copy(
                v_r[:, j, :, 0:HD],
                st["pv"][:, 0:HPC * HD].rearrange("p (h e) -> p h e", e=HD))
        return links, copy

    def po_chain(sc2, half):
        qb_ = sc2 // CPB
        csl = slice((sc2 % CPB) * 128, (sc2 % CPB + 1) * 128)
        w0, w1 = (0, 512) if half == 0 else (512, 768)
        n = w1 - w0
        st = {}
        links = []

        def l0():
            st["po"] = psPR.tile([128, SC], F32, tag="pr", name=f"po{sc2}_{half}")
            MM(st["po"][:, 0:n], ctxAB[qb_][:, csl], woAB_sb[:, w0:w1],
               start=True, stop=False)

        def l1():
            MM(st["po"][:, 0:n], ctxC[qb_][:, csl], woC_sb[:, w0:w1],
               start=False, stop=True)
        links += [l0, l1]
        return links, st

    def outproj_pieces(sc2):
        # two paired 2-link chains (col halves) + cast + dma
        la, sta = po_chain(sc2, 0)
        lb, stb = po_chain(sc2, 1)
        pieces = [lambda: (la[0](), lb[0]()), lambda: (la[1](), lb[1]())]

        def cast_dma():
            ost = ostp.tile([128, 768], BF16, tag="ost", name=f"ost{sc2}")
            nc.vector.tensor_copy(ost[:, 0:512], sta["po"][:, 0:512])
            nc.vector.tensor_copy(ost[:, 512:768], stb["po"][:, 0:256])
            nc.sync.dma_start(out_ap[sc2 * 128:(sc2 + 1) * 128, :], ost[:])
        pieces.append(cast_dma)
        return pieces

    def pair_chains(chain_list):
        # chain_list: [(links, copy), ...] -> pieces, links of adjacent chains
        # interleaved so consecutive matmuls alternate the two psPR banks.
        pieces = []
        for i in range(0, len(chain_list), 2):
            pair = chain_list[i:i + 2]
            if len(pair) == 2:
                (lA, cA), (lB, cB) = pair
                for a, b in zip(lA, lB):
                    pieces.append(lambda a=a, b=b: (a(), b()))
                pieces.append(cA)
                pieces.append(cB)
            else:
                (lA, cA), = pair
                for a in lA:
                    pieces.append(a)
                pieces.append(cA)
        return pieces

    def proj_pieces(sc):
        chains = ([m_chain(sc, m) for m in range(3)] +
                  [v_chain(sc, j) for j in range(CPB)])
        return pair_chains(chains)

    # ---- attention ----
    pend = []
    gctr = [0]
    cps = {}

    def norm_head(qb, h, num, den):
        # reciprocal_approx_fast is a bitwise-seed op and misreads PSUM;
        # bounce the denominator row through SBUF first
        dn = smp.tile([1, SC], F32, tag="dn", name=f"dn{h}_{qb}")
        nc.vector.tensor_copy(dn[:], den)
        rec = smp.tile([1, SC], F32, tag="rec", name=f"rec{h}_{qb}")
        nc.vector.reciprocal_approx_fast(rec[:], dn[:])
        bc = smp.tile([64, SC], F32, tag="bc", name=f"bc{h}_{qb}")
        nc.gpsimd.partition_broadcast(bc[:], rec[:])
        if h == 0:
            nc.vector.tensor_mul(ctxAB[qb][0:64, :], num, bc[:])
        elif h == 1:
            nc.vector.tensor_mul(ctxAB[qb][64:128, :], num, bc[:])
        else:
            nc.vector.tensor_mul(ctxC[qb][:], num, bc[:])

    def emit_pv(weave=True):
        qb, h, ck0, eg, last = pend.pop(0)
        nch = (qb + 1) * CPB
        if ck0 == 0:
            # allocate the ctx accumulator at first PV pop, not at QK emission:
            # the pool ring has 2 bufs and the previous phase's lagged PV
            # writes must all be emitted before its banks are recycled
            cps[h] = psCTX.tile([128, SC], F32, tag="cps", name=f"cps{h}_{qb}")
        for si, ck in enumerate((ck0, ck0 + 1)):
            o = max(0, (ck - (nch - CPB)) * 128)  # causal q-range restriction
            egv = eg[:, si * SC + o:(si + 1) * SC].bitcast(BF16)
            if ck >= nch - CPB:  # diagonal chunk: mask leading cols of view
                nc.gpsimd.affine_select(
                    egv[:, 0:128], egv[:, 0:128], pattern=[[1, 128]],
                    compare_op=GE, fill=0.0, base=0, channel_multiplier=-1)
            v_r = vt[ck // CPB][:].rearrange("p (c h e) -> p c h e",
                                             h=HPC, e=HD + 1)
            MM(cps[h][0:HD + 1, o:SC], v_r[:, ck % CPB, h, :], egv,
               start=(ck == 0), stop=(ck == nch - 1))
            if si == 0 and weave:
                pull()
        if last:
            norm_head(qb, h, cps[h][0:HD, :], cps[h][HD:HD + 1, :])

    def group(qb, h, ck0):
        nch = (qb + 1) * CPB

        def f():
            sg = psSG.tile([128, 2 * SC], F32, tag="sg", name=f"sg{qb}_{h}_{ck0}")
            eg = expp.tile([128, 2 * SC], I16, tag="eg", name=f"eg{qb}_{h}_{ck0}")
            for si, ck in enumerate((ck0, ck0 + 1)):
                o = max(0, (ck - (nch - CPB)) * 128)
                rh = (ck % 2) * 64
                cg = ((ck % CPB) // 2) * 128
                MM(sg[:, si * SC + o:(si + 1) * SC],
                   k2[h][ck // CPB][rh:rh + 64, cg:cg + 128],
                   q2[h][qb][rh:rh + 64, o:SC], start=True, stop=True)
            o0 = max(0, (ck0 - (nch - CPB)) * 128)
            _exp(sg, eg, o0)
            pend.append((qb, h, ck0, eg, ck0 == nch - 2))
            if len(pend) > 2:
                emit_pv()
        return f

    def _exp(sg, eg, o0):
        if gctr[0] % 5 == 4:
            nc.vector.tensor_scalar(eg[:, o0:], sg[:, o0:], A_DVE, B_DVE,
                                    mybir.AluOpType.mult, mybir.AluOpType.add)
        else:
            nc.scalar.activation(eg[:, o0:].bitcast(BF16), sg[:, o0:],
                                 mybir.ActivationFunctionType.Exp, scale=0.125)
        gctr[0] += 1

    # ---- emission ----
    # startup: interleave the wqk/x-chunk-0 DMAs so the first projection
    # chain can begin after the first pair lands; q/k chains first so qb0
    # attention can begin while v chains run.
    xts[0] = []
    for k in range(NKC):
        nc.sync.dma_start(wqk_sb[:, k * 384:(k + 1) * 384],
                          wqk[k * 128:(k + 1) * 128, :])
        xt = xtp.tile([128, SC], BF16, tag="xt", name=f"xt{k}_0")
        nc.sync.dma_start(xt[:], xT[k * 128:(k + 1) * 128, 0:SC])
        xts[0].append(xt)
    mq, cq = m_chain(0, 0)
    mk, ck_ = m_chain(0, 2)
    for piece in pair_chains([(mq, cq), (mk, ck_)]):
        piece()
    for k in range(NKC):
        nc.sync.dma_start(wv_sb[:, k * 256:(k + 1) * 256],
                          wv[k * 128:(k + 1) * 128, :])
    dma_piece(1)()
    nc.sync.dma_start(woAB_sb[:], wo[0:128, :])
    nc.sync.dma_start(woC_sb[:], wo[128:192, :])
    ones_st = smp.tile([128, CPB], F32, tag="ones")
    nc.vector.memset(ones_st[:], 1.0)
    for i in range(NSC):
        v_r = vt[i][:].rearrange("p (c h e) -> p c h e", h=HPC, e=HD + 1)
        for h in range(HPC):
            nc.vector.tensor_copy(v_r[:, :, h, HD], ones_st[:])
    for piece in pair_chains([m_chain(0, 1)] + [v_chain(0, j) for j in range(CPB)]):
        piece()

    for qb in range(NQB):
        nch = (qb + 1) * CPB
        extras = []
        pulled[0] = 0
        if qb + 2 < NSC:
            extras.append(dma_piece(qb + 2))
        if qb + 1 < NSC:
            extras += proj_pieces(qb + 1)
        if qb >= 1:
            for j in range(CPB):
                extras += outproj_pieces((qb - 1) * CPB + j)
        groups = []
        for h in range(HPC):
            for g in range(nch // 2):
                groups.append(group(qb, h, 2 * g))
        n, k0 = len(groups), len(extras)
        for i, gth in enumerate(groups):
            gth()
            due = (i + 1) * k0 // n
            while pulled[0] < due:
                pull()
        while pend:
            emit_pv()
        while pulled[0] < k0:
            pull()

    extras = []
    pulled[0] = 0
    for j in range(CPB):
        extras += outproj_pieces((NQB - 1) * CPB + j)
    for piece in extras:
        piece()

    for p in (psPR, psCTX, psSG, ostp, smp, expp, ctxp, vp, k2p, q2p, xtp, constp):
        p.release()


def _build():
    if "nc" in _CACHE:
        return _CACHE["nc"]
    nc = bacc.Bacc("TRN2", target_bir_lowering=False, debug=False, num_devices=N_CORES)
    xT = nc.dram_tensor("xT", [D, S], BF16, kind="ExternalInput").ap()
    wqk = nc.dram_tensor("wqk", [D, 384], BF16, kind="ExternalInput").ap()
    wv = nc.dram_tensor("wv", [D, 256], BF16, kind="ExternalInput").ap()
    wo = nc.dram_tensor("wo", [HPC * HD, D], BF16, kind="ExternalInput").ap()
    out = nc.dram_tensor("out", [S, D], BF16, kind="ExternalOutput").ap()
    with tile.TileContext(nc) as tc:
        _emit(nc, tc, (xT, wqk, wv, wo), out)
    nc.compile()
    _CACHE["nc"] = nc
    return nc


def _in_maps(x, w_qkv, w_out):
    import ml_dtypes
    xTs = [np.ascontiguousarray(x[b].T).astype(ml_dtypes.bfloat16) for b in range(B)]
    maps = []
    for c in range(N_CORES):
        b = c // 4
        h0 = (c % 4) * HPC
        cols = lambda base, h: w_qkv[:, base + (h0 + h) * HD: base + (h0 + h + 1) * HD]
        wqk = np.ascontiguousarray(np.concatenate(
            [cols(0, 0), cols(0, 1),            # m0: qA | qB
             cols(0, 2), cols(D, 2),            # m1: qC | kC
             cols(D, 0), cols(D, 1)], axis=1)).astype(ml_dtypes.bfloat16)
        wv = np.ascontiguousarray(np.concatenate(
            [cols(2 * D, 0), cols(2 * D, 1), cols(2 * D, 2),
             np.zeros((D, 64), np.float32)], axis=1)).astype(ml_dtypes.bfloat16)
        wo = np.ascontiguousarray(
            w_out[h0 * HD:(h0 + HPC) * HD, :]).astype(ml_dtypes.bfloat16)
        maps.append({"xT": xTs[b], "wqk": wqk, "wv": wv, "wo": wo})
    return maps


def run_sharded(x, w_qkv, w_out, **spmd_kwargs):
    nc = _build()
    res = run_bass_kernel_spmd(nc, _in_maps(x, w_qkv, w_out),
                               list(range(N_CORES)), **spmd_kwargs)
    outs = [res.results[c]["out"].astype(np.float32) for c in range(N_CORES)]
    y = np.empty((B, S, D), np.float32)
    for b in range(B):
        y[b] = outs[4 * b] + outs[4 * b + 1] + outs[4 * b + 2] + outs[4 * b + 3]
    return y, res


def kernel(x, w_qkv, w_out):
    x = np.asarray(x, dtype=np.float32)
    w_qkv = np.asarray(w_qkv, dtype=np.float32)
    w_out = np.asarray(w_out, dtype=np.float32)
    y, _ = run_sharded(x, w_qkv, w_out)
    return y
